# revision 1
# baseline (speedup 1.0000x reference)
"""Trainium2 Bass kernel for nn_AttentionBlock (GroupNorm + 1x1-conv QKV
self-attention + proj + residual), data-parallel over batch across 8 cores.

Math notes (all exactly equivalent to the reference up to fp rounding):
  - GroupNorm folded to per-channel scale/offset: hn = x*scl + off with
    scl = rstd*gamma, off = beta - mean*scl. Group stats come from
    per-channel (sum, sumsq) reduced across the 16 channels of each group
    with a block-diagonal ones matmul (returns group totals per-channel).
  - k bias dropped: softmax((q+bq).(k+bk)) == softmax((q+bq).k) because the
    q.bk and bq.bk terms are constant along the softmax axis.
  - v bias folded into proj bias: rows of softmax sum to 1, so
    proj_w @ (o + bv) + proj_b = proj_w @ o + (proj_w @ bv + proj_b).
  - No max-subtraction in softmax: |scores/sqrt(C)| < ~2 for this data, so
    exp is safe, and softmax is shift-invariant.

Layouts on chip (per sample):
  hn/q/k/o: channel-major [128, 4, 1024]  (partition = channel % 128)
  v: token-major [128, 8, 512]            (computed transposed by swapping
                                           matmul operands; avoids on-chip
                                           transposes entirely)
  pT = exp(scores^T): [128(token m), 8, 512(token n)] per n-half; the
  softmax denominator accumulates on DVE one chunk behind the exps, then a
  single all-ones matmul per half does the cross-partition sum and
  broadcasts it to all 128 PSUM partitions in one shot.

Matmul operands are bf16 by default (stats, softmax normalization, psum
accumulation and the residual epilogue all stay fp32): measured 145.4us /
5.2e-5 relative error vs the fp32 reference on HW. dtype_mode="f32r" keeps
every operand at fp32-storage with the PE's fast rounded mode (~3e-6 error,
a few % slower). Strict fp32 matmuls ("f32") are 4x slower on the PE.

Scheduling: sample 1's GroupNorm statistics are emitted between sample 0's
QKV and attention sections, and its normalize runs on ACT in the idle
window after sample 0's exps (GPSIMD is kept idle during compute — its
SBUF port is exclusively shared with DVE). Big DMAs ride the Sync engine's
hardware DGE queues; Bacc legalizes the multi-queue semaphore waits.
"""

import math
import numpy as np

import concourse.bass as bass
import concourse.bacc as bacc
import concourse.tile as tile
from concourse import bass_isa, mybir
from concourse.bass_utils import run_bass_kernel_spmd

F32 = mybir.dt.float32
F32R = mybir.dt.float32r
AF = mybir.ActivationFunctionType
OP = mybir.AluOpType
AX = mybir.AxisListType

B = 16
C = 512
HW = 1024
NCORES = 8
SPC = B // NCORES          # samples per core
KO = C // 128              # channel chunks of 128
MI = HW // 128             # token chunks of 128
NH = HW // 512             # 512-wide column halves
GSIZE = (C // 32) * HW     # elements per group (16 ch * 1024)
EPS = 1e-5
SM_SCALE = 1.0 / math.sqrt(C)


def build(dtype_mode: str = "f32r") -> bass.Bass:
    nc = bacc.Bacc()
    MD = {"f32r": F32R, "f32": F32, "bf16": mybir.dt.bfloat16}[dtype_mode]

    x_h = nc.declare_dram_parameter("x", [SPC, C, HW], F32, isOutput=False)
    wq_h = nc.declare_dram_parameter("wq", [C, C], MD, isOutput=False)
    wk_h = nc.declare_dram_parameter("wk", [C, C], MD, isOutput=False)
    wv_h = nc.declare_dram_parameter("wv", [C, C], MD, isOutput=False)
    wp_h = nc.declare_dram_parameter("wp", [C, C], MD, isOutput=False)
    bq_h = nc.declare_dram_parameter("bq", [C], F32, isOutput=False)
    pb_h = nc.declare_dram_parameter("pb", [C], F32, isOutput=False)
    gam_h = nc.declare_dram_parameter("gam", [C], F32, isOutput=False)
    bet_h = nc.declare_dram_parameter("bet", [C], F32, isOutput=False)
    gs_h = nc.declare_dram_parameter("gsum", [128, 128], F32, isOutput=False)
    on_h = nc.declare_dram_parameter("ones", [128, 128], F32R, isOutput=False)
    y_h = nc.declare_dram_parameter("y", [SPC, C, HW], F32, isOutput=True)

    with tile.TileContext(nc) as tc:
        with (
            tc.tile_pool(name="const", bufs=1) as const,
            tc.tile_pool(name="xp", bufs=2) as xp,
            tc.tile_pool(name="work", bufs=1) as work,
            tc.tile_pool(name="small", bufs=2) as small,
            tc.tile_pool(name="yp", bufs=3) as yp,
            tc.tile_pool(name="ps1", bufs=2, space="PSUM") as ps1,
            tc.tile_pool(name="ps_s", bufs=3, space="PSUM") as ps_s,
            tc.tile_pool(name="ps_l", bufs=1, space="PSUM") as ps_l,
            tc.tile_pool(name="ps_o", bufs=2, space="PSUM") as ps_o,
        ):
            # x chunk tiles for both samples; sample 0's chunks are DMA'd
            # before the weights so its stats can start immediately
            x_sbs = [[xp.tile([128, HW], F32, tag=f"x{ko}", name=f"x_sb_{s}_{ko}")
                      for ko in range(KO)] for s in range(SPC)]
            for ko in range(KO):
                nc.sync.dma_start(out=x_sbs[0][ko],
                                  in_=x_h[0][ko * 128:(ko + 1) * 128, :])

            # small constants first: the GroupNorm chain needs them long
            # before the big weight tiles are touched
            gs_sb = const.tile([128, 128], F32, tag="gs")
            nc.sync.dma_start(out=gs_sb, in_=gs_h[:])
            bq_sb = const.tile([128, KO], F32, tag="bq")
            nc.sync.dma_start(out=bq_sb, in_=bq_h[:].rearrange("(mo p) -> p mo", p=128))
            pb_sb = const.tile([128, KO], F32, tag="pb")
            nc.sync.dma_start(out=pb_sb, in_=pb_h[:].rearrange("(mo p) -> p mo", p=128))
            gam_sb = const.tile([128, KO], F32, tag="gam")
            nc.sync.dma_start(out=gam_sb, in_=gam_h[:].rearrange("(ko p) -> p ko", p=128))
            bet_sb = const.tile([128, KO], F32, tag="bet")
            nc.sync.dma_start(out=bet_sb, in_=bet_h[:].rearrange("(ko p) -> p ko", p=128))
            ones_sb = const.tile([128, 128], F32R, tag="ones")
            nc.sync.dma_start(out=ones_sb, in_=on_h[:])
            eps_sb = const.tile([128, 1], F32, tag="eps")
            nc.vector.memset(eps_sb, EPS)
            zero_sb = const.tile([128, 1], F32, tag="zero")
            nc.vector.memset(zero_sb, 0.0)
            junk_sb = const.tile([128, HW], F32, tag="junk")
            wq_sb = const.tile([128, KO, C], MD, tag="wq")
            nc.sync.dma_start(out=wq_sb, in_=wq_h[:].rearrange("(ki p) n -> p ki n", p=128))
            wk_sb = const.tile([128, KO, C], MD, tag="wk")
            nc.sync.dma_start(out=wk_sb, in_=wk_h[:].rearrange("(ki p) n -> p ki n", p=128))
            wv_sb = const.tile([128, KO, C], MD, tag="wv")
            nc.sync.dma_start(out=wv_sb, in_=wv_h[:].rearrange("(ki p) n -> p ki n", p=128))
            wp_sb = const.tile([128, KO, C], MD, tag="wp")
            nc.sync.dma_start(out=wp_sb, in_=wp_h[:].rearrange("(ki p) n -> p ki n", p=128))
            # prefetch sample 1
            for ko in range(KO):
                nc.sync.dma_start(out=x_sbs[1][ko],
                                  in_=x_h[1][ko * 128:(ko + 1) * 128, :])

            def emit_gn_stats(s):
                """Per-channel scale/offset for GroupNorm of sample s."""
                x_sb = x_sbs[s]
                st_sb = small.tile([128, KO, 2], F32, tag="st", name=f"st_{s}")
                for ko in range(KO):
                    nc.vector.reduce_sum(out=st_sb[:, ko, 0:1], in_=x_sb[ko], axis=AX.X)
                    # squares land in a scratch tile; only the accumulated
                    # sum-of-squares is kept
                    nc.scalar.activation(
                        out=junk_sb, in_=x_sb[ko],
                        func=AF.Square, bias=zero_sb,
                        accum_out=st_sb[:, ko, 1:2],
                    )
                gps = ps_l.tile([128, KO, 2], F32, tag="lg", name=f"gps_{s}")
                for ko in range(KO):
                    nc.tensor.matmul(gps[:, ko, :], lhsT=gs_sb, rhs=st_sb[:, ko, :],
                                     start=True, stop=True)
                # gsum is pre-scaled by 1/GSIZE on the host, so gps holds
                # [mean, E[x^2]] directly. mean^2 runs on DVE (not ACT
                # Square) so the Square->Sqrt activation-table swap happens
                # early, off this latency-critical chain.
                mean_sb = small.tile([128, KO], F32, tag="mean", name=f"mean_{s}")
                nc.vector.tensor_copy(out=mean_sb, in_=gps[:, :, 0])
                msq_sb = small.tile([128, KO], F32, tag="msq", name=f"msq_{s}")
                nc.vector.tensor_mul(msq_sb, mean_sb, mean_sb)
                var_sb = small.tile([128, KO], F32, tag="var", name=f"var_{s}")
                nc.vector.tensor_sub(var_sb, gps[:, :, 1], msq_sb)
                std_sb = small.tile([128, KO], F32, tag="std", name=f"std_{s}")
                nc.scalar.activation(out=std_sb, in_=var_sb, func=AF.Sqrt, bias=eps_sb)
                rstd_sb = small.tile([128, KO], F32, tag="rstd", name=f"rstd_{s}")
                nc.vector.reciprocal_approx_fast(out=rstd_sb, in_=std_sb)
                scl_sb = small.tile([128, KO], F32, tag="scl", name=f"scl_{s}")
                nc.vector.tensor_mul(scl_sb, rstd_sb, gam_sb)
                off_sb = small.tile([128, KO], F32, tag="off", name=f"off_{s}")
                nc.vector.tensor_mul(off_sb, mean_sb, scl_sb)
                nc.vector.tensor_sub(off_sb, bet_sb, off_sb)
                return scl_sb, off_sb

            def emit_gn_norm(s, scl_sb, off_sb, spread):
                """hn = x*scl + off. spread=True splits chunks across
                DVE/ACT/GPSIMD to minimize latency (sample 0's critical
                path); spread=False keeps it all on GPSIMD so it can hide
                under the previous sample's attention."""
                hn_sb = work.tile([128, KO, HW], MD, tag="hn", name=f"hn_{s}")
                for ko in range(KO):
                    # no GPSIMD here: its SBUF port is shared (exclusively
                    # locked) with DVE, and any POOL activity slows the
                    # attention-critical DVE stream
                    eng = ["dve", "act", "act", "dve"][ko] if spread else "act"
                    if eng == "act":
                        nc.scalar.activation(
                            out=hn_sb[:, ko, :], in_=x_sbs[s][ko],
                            func=AF.Identity, bias=off_sb[:, ko:ko + 1],
                            scale=scl_sb[:, ko:ko + 1])
                    else:
                        e = nc.vector if eng == "dve" else nc.gpsimd
                        e.tensor_scalar(
                            out=hn_sb[:, ko, :], in0=x_sbs[s][ko],
                            scalar1=scl_sb[:, ko:ko + 1], scalar2=off_sb[:, ko:ko + 1],
                            op0=OP.mult, op1=OP.add,
                        )
                return hn_sb

            def emit_qkv(s, hn_sb):
                q_sb = work.tile([128, KO, HW], MD, tag="q", name=f"q_{s}")
                k_sb = work.tile([128, KO, HW], MD, tag="k", name=f"k_{s}")
                v_sb = work.tile([128, MI, C], MD, tag="v", name=f"v_{s}")
                for mo in range(KO):
                    for nh in range(NH):
                        pq = ps1.tile([128, 512], F32, tag="pmm", name="pq")
                        for ki in range(KO):
                            nc.tensor.matmul(
                                pq, lhsT=wq_sb[:, ki, mo * 128:(mo + 1) * 128],
                                rhs=hn_sb[:, ki, nh * 512:(nh + 1) * 512],
                                start=(ki == 0), stop=(ki == KO - 1))
                        nc.vector.tensor_scalar_add(
                            out=q_sb[:, mo, nh * 512:(nh + 1) * 512], in0=pq,
                            scalar1=bq_sb[:, mo:mo + 1])
                    for nh in range(NH):
                        pk = ps1.tile([128, 512], F32, tag="pmm", name="pk")
                        for ki in range(KO):
                            nc.tensor.matmul(
                                pk, lhsT=wk_sb[:, ki, mo * 128:(mo + 1) * 128],
                                rhs=hn_sb[:, ki, nh * 512:(nh + 1) * 512],
                                start=(ki == 0), stop=(ki == KO - 1))
                        nc.scalar.copy(out=k_sb[:, mo, nh * 512:(nh + 1) * 512], in_=pk)
                for mi in range(MI):
                    pv = ps1.tile([128, 512], F32, tag="pmm", name="pv")
                    for ki in range(KO):
                        nc.tensor.matmul(
                            pv, lhsT=hn_sb[:, ki, mi * 128:(mi + 1) * 128],
                            rhs=wv_sb[:, ki, :],
                            start=(ki == 0), stop=(ki == KO - 1))
                    nc.vector.tensor_copy(out=v_sb[:, mi, :], in_=pv)
                return q_sb, k_sb, v_sb

            def emit_attention(s, q_sb, k_sb, v_sb):
                o_sb = work.tile([128, KO, HW], MD, tag="o", name=f"o_{s}")
                for nh in range(NH):
                    nsl = slice(nh * 512, (nh + 1) * 512)
                    pT_sb = work.tile([128, MI, 512], MD, tag="pT", name=f"pT_{s}_{nh}")
                    lac_sb = small.tile([128, 512], F32R, tag="lac", name=f"lac_{s}_{nh}")
                    for mi in range(MI):
                        sps = ps_s.tile([128, 512], F32, tag="s", name="sps")
                        for ki in range(KO):
                            nc.tensor.matmul(
                                sps, lhsT=k_sb[:, ki, mi * 128:(mi + 1) * 128],
                                rhs=q_sb[:, ki, nsl],
                                start=(ki == 0), stop=(ki == KO - 1))
                        nc.scalar.activation(out=pT_sb[:, mi, :], in_=sps,
                                             func=AF.Exp, bias=zero_sb,
                                             scale=SM_SCALE)
                        # accumulate the softmax denominator on DVE, one chunk
                        # behind the exps; the cross-partition sum happens once
                        # on GPSIMD below (keeps it all off the PE)
                        if mi == 1:
                            nc.vector.tensor_add(lac_sb, pT_sb[:, 0, :], pT_sb[:, 1, :])
                        elif mi > 1:
                            nc.vector.tensor_add(lac_sb, lac_sb, pT_sb[:, mi, :])
                    rbc_sb = small.tile([128, 512], F32, tag="rbc", name=f"rbc_{s}_{nh}")
                    for co in range(KO):
                        ops = ps_o.tile([128, 512], F32, tag="ops", name="ops")
                        for mi in range(MI):
                            nc.tensor.matmul(
                                ops, lhsT=v_sb[:, mi, co * 128:(co + 1) * 128],
                                rhs=pT_sb[:, mi, :],
                                start=(mi == 0), stop=(mi == MI - 1))
                        if co == 0:
                            # denominator reduce+broadcast rides behind the
                            # first o-group so its wait on the DVE accumulator
                            # hides under real PE work
                            lbc = ps_l.tile([128, 512], F32, tag="lg",
                                            name=f"lbc_{s}_{nh}")
                            nc.tensor.matmul(lbc, lhsT=ones_sb, rhs=lac_sb,
                                             start=True, stop=True)
                            nc.vector.reciprocal_approx_fast(out=rbc_sb, in_=lbc)
                        nc.vector.tensor_mul(o_sb[:, co, nsl], ops, rbc_sb)
                return o_sb

            def emit_proj(s, o_sb):
                for co in range(KO):
                    for nh in range(NH):
                        pp = ps1.tile([128, 512], F32, tag="pmm", name="pp")
                        for ki in range(KO):
                            nc.tensor.matmul(
                                pp, lhsT=wp_sb[:, ki, co * 128:(co + 1) * 128],
                                rhs=o_sb[:, ki, nh * 512:(nh + 1) * 512],
                                start=(ki == 0), stop=(ki == KO - 1))
                        y_sb = yp.tile([128, 512], F32, tag="y", name="y_sb")
                        nc.vector.scalar_tensor_tensor(
                            out=y_sb, in0=pp, scalar=pb_sb[:, co:co + 1],
                            in1=x_sbs[s][co][:, nh * 512:(nh + 1) * 512],
                            op0=OP.add, op1=OP.add)
                        nc.sync.dma_start(
                            out=y_h[s][co * 128:(co + 1) * 128,
                                       nh * 512:(nh + 1) * 512],
                            in_=y_sb)

            # software-pipelined schedule over the two samples
            scl0, off0 = emit_gn_stats(0)
            hn0 = emit_gn_norm(0, scl0, off0, spread=True)
            qkv0 = emit_qkv(0, hn0)
            # sample 1 stats ride the idle DVE/ACT time under sample 0's
            # attention; its normalize starts the moment hn's buffer frees
            scl1, off1 = emit_gn_stats(1)
            o0 = emit_attention(0, *qkv0)
            # normalize for sample 1 lands in ACT's idle window during
            # sample 0's o/proj matmuls, after all of its exps
            hn1 = emit_gn_norm(1, scl1, off1, spread=False)
            emit_proj(0, o0)
            qkv1 = emit_qkv(1, hn1)
            o1 = emit_attention(1, *qkv1)
            emit_proj(1, o1)

    nc.compile()
    return nc


_NC_CACHE: dict = {}


def _get_nc(dtype_mode: str = "f32r") -> bass.Bass:
    if dtype_mode not in _NC_CACHE:
        _NC_CACHE[dtype_mode] = build(dtype_mode)
    return _NC_CACHE[dtype_mode]


def make_in_maps(x, gamma, beta, qkv_w, qkv_b, proj_w, proj_b, dtype_mode="f32r"):
    f32 = np.float32
    x = np.ascontiguousarray(np.asarray(x, dtype=f32)).reshape(B, C, HW)
    qkv_w = np.asarray(qkv_w, dtype=f32)
    qkv_b = np.asarray(qkv_b, dtype=f32)
    proj_w = np.asarray(proj_w, dtype=f32)
    proj_b = np.asarray(proj_b, dtype=f32)
    shared = {
        "wq": np.ascontiguousarray(qkv_w[0:C].T),
        "wk": np.ascontiguousarray(qkv_w[C:2 * C].T),
        "wv": np.ascontiguousarray(qkv_w[2 * C:3 * C].T),
        "wp": np.ascontiguousarray(proj_w.T),
        "bq": np.ascontiguousarray(qkv_b[0:C]),
        "pb": (proj_w.astype(np.float64) @ qkv_b[2 * C:3 * C].astype(np.float64)
               + proj_b.astype(np.float64)).astype(f32),
        "gam": np.ascontiguousarray(np.asarray(gamma, dtype=f32)),
        "bet": np.ascontiguousarray(np.asarray(beta, dtype=f32)),
        "gsum": np.kron(np.eye(8, dtype=f32), np.ones((16, 16), dtype=f32)) * f32(1.0 / GSIZE),
        "ones": np.ones((128, 128), dtype=f32),
    }
    if dtype_mode == "bf16":
        import ml_dtypes
        bf16 = np.dtype(ml_dtypes.bfloat16)
        for k in ("wq", "wk", "wv", "wp"):
            shared[k] = shared[k].astype(bf16)
    return [dict(shared, x=np.ascontiguousarray(x[i * SPC:(i + 1) * SPC]))
            for i in range(NCORES)]


def run(x, gamma, beta, qkv_w, qkv_b, proj_w, proj_b, trace=False, dtype_mode="bf16"):
    in_maps = make_in_maps(x, gamma, beta, qkv_w, qkv_b, proj_w, proj_b, dtype_mode)
    nc = _get_nc(dtype_mode)
    res = run_bass_kernel_spmd(nc, in_maps, list(range(NCORES)), trace=trace)
    y = np.concatenate([res.results[i]["y"] for i in range(NCORES)], axis=0)
    return y.reshape(B, C, 32, 32).astype(np.float32), res


def kernel(**inputs) -> np.ndarray:
    y, _ = run(**inputs)
    return y



# revision 5
# speedup vs baseline: 1.3563x; 1.3563x over previous
"""Trainium2 Bass kernel for nn_AttentionBlock (GroupNorm + 1x1-conv QKV
self-attention + proj + residual), data-parallel over batch across 8 cores.

Math notes (all exactly equivalent to the reference up to fp rounding):
  - GroupNorm folded to per-channel scale/offset: hn = x*scl + off with
    scl = rstd*gamma, off = beta - mean*scl. Group stats come from
    per-channel (sum, sumsq) reduced across the 16 channels of each group
    with a block-diagonal ones matmul (returns group totals per-channel).
  - k bias dropped: softmax((q+bq).(k+bk)) == softmax((q+bq).k) because the
    q.bk and bq.bk terms are constant along the softmax axis.
  - v bias folded into proj bias: rows of softmax sum to 1, so
    proj_w @ (o + bv) + proj_b = proj_w @ o + (proj_w @ bv + proj_b).
  - No max-subtraction in softmax: |scores/sqrt(C)| < ~2 for this data, so
    exp is safe, and softmax is shift-invariant.
  - Softmax normalization deferred past proj: the denominator is per-token
    (per output column), so it commutes with the channel contraction.
    o stays unnormalized through the attn@V and proj matmuls; the epilogue
    multiplies by 1/denom. |o_unnorm| < ~120, inside fp8e4's +-240 range.

All six big matmul families (QKV, scores, attn@V, proj, and the softmax
denominator) run in fp8e4 with perf_mode=DoubleRow: operands are 3D APs
[128, 2, free] contracting 256 logical K per instruction, doubling PE
throughput vs bf16. Accumulation, GroupNorm stats, the denominator
reciprocal and the residual epilogue all stay fp32. Measured relative
error vs the fp32 reference ~8e-4 (numpy bit-exact simulation ~8.4e-4).

Layouts on chip (per sample):
  hn/q/k/o: channel-major [128, 4, 1024]  (partition = channel % 128)
  v: token-major [128, 8, 512]            (computed transposed by swapping
                                           matmul operands; avoids on-chip
                                           transposes entirely)
  pT = exp(scores^T): [128(token m), 8, 1024(token n)], fp8. The softmax
  denominator is an fp8 all-ones DoubleRow matmul over pT (4 MMs per
  512-wide half) that reduces across tokens AND broadcasts the sums to all
  128 PSUM partitions in one shot; its reciprocal is only needed at the
  proj epilogue, far off the critical path.

Scheduling: sample 1's GroupNorm statistics are emitted between sample 0's
QKV and attention sections, and its normalize runs on ACT in the idle
window after sample 0's exps (GPSIMD is kept idle during compute -- its
SBUF port is exclusively shared with DVE). Big DMAs ride the Sync engine's
hardware DGE queues; Bacc legalizes the multi-queue semaphore waits.
"""

import math
import numpy as np

import concourse.bass as bass
import concourse.bacc as bacc
import concourse.tile as tile
from concourse import bass_isa, mybir
from concourse.bass_utils import run_bass_kernel_spmd

F32 = mybir.dt.float32
F32R = mybir.dt.float32r
FP8 = mybir.dt.float8e4
AF = mybir.ActivationFunctionType
OP = mybir.AluOpType
AX = mybir.AxisListType
DR = mybir.MatmulPerfMode.DoubleRow

B = 16
C = 512
HW = 1024
NCORES = 8
SPC = B // NCORES          # samples per core
KO = C // 128              # channel chunks of 128
KP = KO // 2               # channel pair-chunks (256-deep DoubleRow)
MI = HW // 128             # token chunks of 128
MP = MI // 2               # token pair-chunks
NH = HW // 512             # 512-wide column halves
GSIZE = (C // 32) * HW     # elements per group (16 ch * 1024)
EPS = 1e-5
SM_SCALE = 1.0 / math.sqrt(C)


def build() -> bass.Bass:
    nc = bacc.Bacc()

    x_h = nc.declare_dram_parameter("x", [SPC, C, HW], F32, isOutput=False)
    wq_h = nc.declare_dram_parameter("wq", [C, C], FP8, isOutput=False)
    wk_h = nc.declare_dram_parameter("wk", [C, C], FP8, isOutput=False)
    wv_h = nc.declare_dram_parameter("wv", [C, C], FP8, isOutput=False)
    wp_h = nc.declare_dram_parameter("wp", [C, C], FP8, isOutput=False)
    bq_h = nc.declare_dram_parameter("bq", [C], F32, isOutput=False)
    pb_h = nc.declare_dram_parameter("pb", [C], F32, isOutput=False)
    gam_h = nc.declare_dram_parameter("gam", [C], F32, isOutput=False)
    bet_h = nc.declare_dram_parameter("bet", [C], F32, isOutput=False)
    gs_h = nc.declare_dram_parameter("gsum", [128, 128], F32, isOutput=False)
    on_h = nc.declare_dram_parameter("ones8", [128, 2, 128], FP8, isOutput=False)
    y_h = nc.declare_dram_parameter("y", [SPC, C, HW], F32, isOutput=True)

    with tile.TileContext(nc) as tc:
        with (
            tc.tile_pool(name="const", bufs=1) as const,
            tc.tile_pool(name="xp", bufs=2) as xp,
            tc.tile_pool(name="work", bufs=2) as work,
            tc.tile_pool(name="small", bufs=2) as small,
            tc.tile_pool(name="yp", bufs=3) as yp,
            tc.tile_pool(name="ps1", bufs=4, space="PSUM") as ps1,
            tc.tile_pool(name="ps_s", bufs=3, space="PSUM") as ps_s,
            tc.tile_pool(name="ps_l", bufs=1, space="PSUM") as ps_l,
        ):
            # x chunk tiles for both samples; sample 0's chunks are DMA'd
            # before the weights so its stats can start immediately
            x_sbs = [[xp.tile([128, HW], F32, tag=f"x{ko}", name=f"x_sb_{s}_{ko}")
                      for ko in range(KO)] for s in range(SPC)]
            for ko in range(KO):
                nc.sync.dma_start(out=x_sbs[0][ko],
                                  in_=x_h[0][ko * 128:(ko + 1) * 128, :])

            # small constants first: the GroupNorm chain needs them long
            # before the big weight tiles are touched
            gs_sb = const.tile([128, 128], F32, tag="gs")
            nc.sync.dma_start(out=gs_sb, in_=gs_h[:])
            bq_sb = const.tile([128, KO], F32, tag="bq")
            nc.sync.dma_start(out=bq_sb, in_=bq_h[:].rearrange("(mo p) -> p mo", p=128))
            pb_sb = const.tile([128, KO], F32, tag="pb")
            nc.sync.dma_start(out=pb_sb, in_=pb_h[:].rearrange("(mo p) -> p mo", p=128))
            gam_sb = const.tile([128, KO], F32, tag="gam")
            nc.sync.dma_start(out=gam_sb, in_=gam_h[:].rearrange("(ko p) -> p ko", p=128))
            bet_sb = const.tile([128, KO], F32, tag="bet")
            nc.sync.dma_start(out=bet_sb, in_=bet_h[:].rearrange("(ko p) -> p ko", p=128))
            ones8_sb = const.tile([128, 2, 128], FP8, tag="ones8")
            nc.sync.dma_start(out=ones8_sb, in_=on_h[:])
            eps_sb = const.tile([128, 1], F32, tag="eps")
            nc.vector.memset(eps_sb, EPS)
            zero_sb = const.tile([128, 1], F32, tag="zero")
            nc.vector.memset(zero_sb, 0.0)
            junk_sb = const.tile([128, HW], F32, tag="junk")
            wq_sb = const.tile([128, KO, C], FP8, tag="wq")
            nc.sync.dma_start(out=wq_sb, in_=wq_h[:].rearrange("(ki p) n -> p ki n", p=128))
            wk_sb = const.tile([128, KO, C], FP8, tag="wk")
            nc.sync.dma_start(out=wk_sb, in_=wk_h[:].rearrange("(ki p) n -> p ki n", p=128))
            wv_sb = const.tile([128, KO, C], FP8, tag="wv")
            nc.sync.dma_start(out=wv_sb, in_=wv_h[:].rearrange("(ki p) n -> p ki n", p=128))
            wp_sb = const.tile([128, KO, C], FP8, tag="wp")
            nc.sync.dma_start(out=wp_sb, in_=wp_h[:].rearrange("(ki p) n -> p ki n", p=128))
            # prefetch sample 1
            for ko in range(KO):
                nc.sync.dma_start(out=x_sbs[1][ko],
                                  in_=x_h[1][ko * 128:(ko + 1) * 128, :])

            def emit_gn_stats(s):
                """Per-channel scale/offset for GroupNorm of sample s."""
                x_sb = x_sbs[s]
                st_sb = small.tile([128, KO, 2], F32, tag="st", name=f"st_{s}")
                for ko in range(KO):
                    nc.vector.reduce_sum(out=st_sb[:, ko, 0:1], in_=x_sb[ko], axis=AX.X)
                    # squares land in a scratch tile; only the accumulated
                    # sum-of-squares is kept
                    nc.scalar.activation(
                        out=junk_sb, in_=x_sb[ko],
                        func=AF.Square, bias=zero_sb,
                        accum_out=st_sb[:, ko, 1:2],
                    )
                gps = ps_s.tile([128, KO, 2], F32, tag="s", name=f"gps_{s}")
                for ko in range(KO):
                    nc.tensor.matmul(gps[:, ko, :], lhsT=gs_sb, rhs=st_sb[:, ko, :],
                                     start=True, stop=True)
                # gsum is pre-scaled by 1/GSIZE on the host, so gps holds
                # [mean, E[x^2]] directly. mean^2 runs on DVE (not ACT
                # Square) so the Square->Sqrt activation-table swap happens
                # early, off this latency-critical chain.
                mean_sb = small.tile([128, KO], F32, tag="mean", name=f"mean_{s}")
                nc.vector.tensor_copy(out=mean_sb, in_=gps[:, :, 0])
                msq_sb = small.tile([128, KO], F32, tag="msq", name=f"msq_{s}")
                nc.vector.tensor_mul(msq_sb, mean_sb, mean_sb)
                var_sb = small.tile([128, KO], F32, tag="var", name=f"var_{s}")
                nc.vector.tensor_sub(var_sb, gps[:, :, 1], msq_sb)
                std_sb = small.tile([128, KO], F32, tag="std", name=f"std_{s}")
                nc.scalar.activation(out=std_sb, in_=var_sb, func=AF.Sqrt, bias=eps_sb)
                rstd_sb = small.tile([128, KO], F32, tag="rstd", name=f"rstd_{s}")
                nc.vector.reciprocal_approx_fast(out=rstd_sb, in_=std_sb)
                scl_sb = small.tile([128, KO], F32, tag="scl", name=f"scl_{s}")
                nc.vector.tensor_mul(scl_sb, rstd_sb, gam_sb)
                off_sb = small.tile([128, KO], F32, tag="off", name=f"off_{s}")
                nc.vector.tensor_mul(off_sb, mean_sb, scl_sb)
                nc.vector.tensor_sub(off_sb, bet_sb, off_sb)
                return scl_sb, off_sb

            def emit_gn_norm(s, scl_sb, off_sb, spread):
                """hn = x*scl + off. spread=True splits chunks across
                DVE/ACT to minimize latency (sample 0's critical path);
                spread=False keeps it all on ACT so it can hide under the
                previous sample's attention."""
                hn_sb = work.tile([128, KO, HW], FP8, tag="hn", name=f"hn_{s}")
                for ko in range(KO):
                    # no GPSIMD here: its SBUF port is shared (exclusively
                    # locked) with DVE, and any POOL activity slows the
                    # attention-critical DVE stream
                    eng = ["dve", "act", "act", "dve"][ko] if spread else "act"
                    if eng == "act":
                        nc.scalar.activation(
                            out=hn_sb[:, ko, :], in_=x_sbs[s][ko],
                            func=AF.Identity, bias=off_sb[:, ko:ko + 1],
                            scale=scl_sb[:, ko:ko + 1])
                    else:
                        nc.vector.tensor_scalar(
                            out=hn_sb[:, ko, :], in0=x_sbs[s][ko],
                            scalar1=scl_sb[:, ko:ko + 1], scalar2=off_sb[:, ko:ko + 1],
                            op0=OP.mult, op1=OP.add,
                        )
                return hn_sb

            def emit_qkv(s, hn_sb):
                q_sb = work.tile([128, KO, HW], FP8, tag="q", name=f"q_{s}")
                k_sb = work.tile([128, KO, HW], FP8, tag="k", name=f"k_{s}")
                v_sb = work.tile([128, MI, C], FP8, tag="v", name=f"v_{s}")
                for mo in range(KO):
                    msl = slice(mo * 128, (mo + 1) * 128)
                    pq = [ps1.tile([128, 512], F32, tag="pmm", name=f"pq{nh}")
                          for nh in range(NH)]
                    for j in range(KP):
                        for nh in range(NH):
                            nc.tensor.matmul(
                                pq[nh], lhsT=wq_sb[:, 2 * j:2 * j + 2, msl],
                                rhs=hn_sb[:, 2 * j:2 * j + 2, nh * 512:(nh + 1) * 512],
                                start=(j == 0), stop=(j == KP - 1), perf_mode=DR)
                    for nh in range(NH):
                        nc.vector.tensor_scalar_add(
                            out=q_sb[:, mo, nh * 512:(nh + 1) * 512], in0=pq[nh],
                            scalar1=bq_sb[:, mo:mo + 1])
                    pk = [ps1.tile([128, 512], F32, tag="pmm", name=f"pk{nh}")
                          for nh in range(NH)]
                    for j in range(KP):
                        for nh in range(NH):
                            nc.tensor.matmul(
                                pk[nh], lhsT=wk_sb[:, 2 * j:2 * j + 2, msl],
                                rhs=hn_sb[:, 2 * j:2 * j + 2, nh * 512:(nh + 1) * 512],
                                start=(j == 0), stop=(j == KP - 1), perf_mode=DR)
                    for nh in range(NH):
                        nc.scalar.copy(out=k_sb[:, mo, nh * 512:(nh + 1) * 512],
                                       in_=pk[nh])
                for mi in range(MI):
                    pv = ps1.tile([128, 512], F32, tag="pmm", name="pv")
                    for j in range(KP):
                        nc.tensor.matmul(
                            pv, lhsT=hn_sb[:, 2 * j:2 * j + 2, mi * 128:(mi + 1) * 128],
                            rhs=wv_sb[:, 2 * j:2 * j + 2, :],
                            start=(j == 0), stop=(j == KP - 1), perf_mode=DR)
                    nc.vector.tensor_copy(out=v_sb[:, mi, :], in_=pv)
                return q_sb, k_sb, v_sb

            def emit_attention(s, q_sb, k_sb, v_sb):
                pT_sb = work.tile([128, MI, HW], FP8, tag="pT", name=f"pT_{s}")
                rbc_sb = small.tile([128, HW], F32, tag="rbc", name=f"rbc_{s}")
                for mi in range(MI):
                    sps = [ps_s.tile([128, 512], F32, tag="s", name=f"sps{nh}")
                           for nh in range(NH)]
                    for j in range(KP):
                        for nh in range(NH):
                            nc.tensor.matmul(
                                sps[nh], lhsT=k_sb[:, 2 * j:2 * j + 2,
                                                   mi * 128:(mi + 1) * 128],
                                rhs=q_sb[:, 2 * j:2 * j + 2, nh * 512:(nh + 1) * 512],
                                start=(j == 0), stop=(j == KP - 1), perf_mode=DR)
                    for nh in range(NH):
                        nc.scalar.activation(out=pT_sb[:, mi, nh * 512:(nh + 1) * 512],
                                             in_=sps[nh], func=AF.Exp, bias=zero_sb,
                                             scale=SM_SCALE)
                o_sb = work.tile([128, KO, HW], FP8, tag="o", name=f"o_{s}")
                for co in range(KO):
                    if co < NH:
                        # softmax denominator for column-half `co`: an fp8
                        # all-ones DoubleRow matmul reduces pT across tokens
                        # and broadcasts to all 128 PSUM partitions. Emitted
                        # ahead of this o-group so its reciprocal hides
                        # under the group's PE work.
                        dps = ps_l.tile([128, 512], F32, tag="lg", name=f"dps{co}_{s}")
                        for u in range(MP):
                            nc.tensor.matmul(
                                dps, lhsT=ones8_sb,
                                rhs=pT_sb[:, 2 * u:2 * u + 2, co * 512:(co + 1) * 512],
                                start=(u == 0), stop=(u == MP - 1), perf_mode=DR)
                    ops = [ps1.tile([128, 512], F32, tag="pmm", name=f"ops{nh}")
                           for nh in range(NH)]
                    for u in range(MP):
                        for nh in range(NH):
                            nc.tensor.matmul(
                                ops[nh], lhsT=v_sb[:, 2 * u:2 * u + 2,
                                                   co * 128:(co + 1) * 128],
                                rhs=pT_sb[:, 2 * u:2 * u + 2, nh * 512:(nh + 1) * 512],
                                start=(u == 0), stop=(u == MP - 1), perf_mode=DR)
                    if co < NH:
                        nc.vector.reciprocal_approx_fast(
                            out=rbc_sb[:, co * 512:(co + 1) * 512], in_=dps)
                    # o stays unnormalized (normalization folded into the
                    # proj epilogue); split psum drains across DVE and ACT
                    nc.vector.tensor_copy(out=o_sb[:, co, 0:512], in_=ops[0])
                    nc.scalar.copy(out=o_sb[:, co, 512:1024], in_=ops[1])
                return o_sb, rbc_sb

            def emit_proj(s, o_sb, rbc_sb):
                for co in range(KO):
                    for nh in range(NH):
                        pp = ps1.tile([128, 512], F32, tag="pmm", name="pp")
                        for j in range(KP):
                            nc.tensor.matmul(
                                pp, lhsT=wp_sb[:, 2 * j:2 * j + 2,
                                               co * 128:(co + 1) * 128],
                                rhs=o_sb[:, 2 * j:2 * j + 2, nh * 512:(nh + 1) * 512],
                                start=(j == 0), stop=(j == KP - 1), perf_mode=DR)
                        t_sb = yp.tile([128, 512], F32, tag="t", name="t_sb")
                        nc.vector.tensor_mul(t_sb, pp,
                                             rbc_sb[:, nh * 512:(nh + 1) * 512])
                        y_sb = yp.tile([128, 512], F32, tag="y", name="y_sb")
                        nc.vector.scalar_tensor_tensor(
                            out=y_sb, in0=t_sb, scalar=pb_sb[:, co:co + 1],
                            in1=x_sbs[s][co][:, nh * 512:(nh + 1) * 512],
                            op0=OP.add, op1=OP.add)
                        nc.sync.dma_start(
                            out=y_h[s][co * 128:(co + 1) * 128,
                                       nh * 512:(nh + 1) * 512],
                            in_=y_sb)

            # software-pipelined schedule over the two samples
            scl0, off0 = emit_gn_stats(0)
            hn0 = emit_gn_norm(0, scl0, off0, spread=True)
            qkv0 = emit_qkv(0, hn0)
            # sample 1 stats ride the idle DVE/ACT time under sample 0's
            # attention; its normalize starts the moment hn's buffer frees
            scl1, off1 = emit_gn_stats(1)
            o0, rbc0 = emit_attention(0, *qkv0)
            # normalize for sample 1 lands in ACT's idle window during
            # sample 0's o/proj matmuls, after all of its exps
            hn1 = emit_gn_norm(1, scl1, off1, spread=False)
            emit_proj(0, o0, rbc0)
            qkv1 = emit_qkv(1, hn1)
            o1, rbc1 = emit_attention(1, *qkv1)
            emit_proj(1, o1, rbc1)

    nc.compile()
    return nc


_NC_CACHE: dict = {}


def _get_nc() -> bass.Bass:
    if "fp8" not in _NC_CACHE:
        _NC_CACHE["fp8"] = build()
    return _NC_CACHE["fp8"]


def make_in_maps(x, gamma, beta, qkv_w, qkv_b, proj_w, proj_b):
    import ml_dtypes
    f32 = np.float32
    fp8 = np.dtype(ml_dtypes.float8_e4m3)
    x = np.ascontiguousarray(np.asarray(x, dtype=f32)).reshape(B, C, HW)
    qkv_w = np.asarray(qkv_w, dtype=f32)
    qkv_b = np.asarray(qkv_b, dtype=f32)
    proj_w = np.asarray(proj_w, dtype=f32)
    proj_b = np.asarray(proj_b, dtype=f32)
    shared = {
        "wq": np.ascontiguousarray(qkv_w[0:C].T).astype(fp8),
        "wk": np.ascontiguousarray(qkv_w[C:2 * C].T).astype(fp8),
        "wv": np.ascontiguousarray(qkv_w[2 * C:3 * C].T).astype(fp8),
        "wp": np.ascontiguousarray(proj_w.T).astype(fp8),
        "bq": np.ascontiguousarray(qkv_b[0:C]),
        "pb": (proj_w.astype(np.float64) @ qkv_b[2 * C:3 * C].astype(np.float64)
               + proj_b.astype(np.float64)).astype(f32),
        "gam": np.ascontiguousarray(np.asarray(gamma, dtype=f32)),
        "bet": np.ascontiguousarray(np.asarray(beta, dtype=f32)),
        "gsum": np.kron(np.eye(8, dtype=f32), np.ones((16, 16), dtype=f32)) * f32(1.0 / GSIZE),
        "ones8": np.ones((128, 2, 128), dtype=fp8),
    }
    return [dict(shared, x=np.ascontiguousarray(x[i * SPC:(i + 1) * SPC]))
            for i in range(NCORES)]


def run(x, gamma, beta, qkv_w, qkv_b, proj_w, proj_b, trace=False, dtype_mode="fp8"):
    in_maps = make_in_maps(x, gamma, beta, qkv_w, qkv_b, proj_w, proj_b)
    nc = _get_nc()
    res = run_bass_kernel_spmd(nc, in_maps, list(range(NCORES)), trace=trace)
    y = np.concatenate([res.results[i]["y"] for i in range(NCORES)], axis=0)
    return y.reshape(B, C, 32, 32).astype(np.float32), res


def kernel(**inputs) -> np.ndarray:
    y, _ = run(**inputs)
    return y


# revision 17
# speedup vs baseline: 1.3677x; 1.0085x over previous
"""Trainium2 Bass kernel for nn_AttentionBlock (GroupNorm + 1x1-conv QKV
self-attention + proj + residual), data-parallel over batch across 8 cores.

Math notes (all exactly equivalent to the reference up to fp rounding):
  - GroupNorm folded to per-channel scale/offset: hn = x*scl + off with
    scl = rstd*gamma, off = beta - mean*scl. Group stats come from
    per-channel (sum, sumsq) reduced across the 16 channels of each group
    with a block-diagonal ones matmul (returns group totals per-channel).
  - rstd computed as exp(-0.5*ln(var+eps)) instead of 1/sqrt: Ln and Exp
    live in the same ACT table set (natural_log_exp_and_others) as
    Square/Identity/Copy, so the whole kernel needs exactly ONE
    ACT_TABLE_LOAD (a Sqrt would force ~2.7us table swaps per sample).
  - k bias dropped: softmax((q+bq).(k+bk)) == softmax((q+bq).k) because the
    q.bk and bq.bk terms are constant along the softmax axis.
  - v bias folded into proj bias: rows of softmax sum to 1, so
    proj_w @ (o + bv) + proj_b = proj_w @ o + (proj_w @ bv + proj_b).
  - No max-subtraction in softmax: |scores/sqrt(C)| < ~2 for this data, so
    exp is safe, and softmax is shift-invariant.

All six big matmul families (QKV, scores, attn@V, proj, and the softmax
denominator) run in fp8e4 with perf_mode=DoubleRow: operands are 3D APs
[128, 2, free] contracting 256 logical K per instruction, doubling PE
throughput vs bf16 (measured 216 ns issue-to-issue for N=512). The softmax
denominator is an fp8 all-ones DoubleRow matmul over pT that reduces
across tokens AND broadcasts the sums to all 128 PSUM partitions in one
shot. Accumulation, GroupNorm stats, reciprocals and the residual epilogue
stay fp32. Measured relative error vs the fp32 reference ~8e-4.

Engine economics (errata-adjusted): DVE costs (120+FD)/0.96 ns from PSUM,
ACT (172+FD)/1.2, so every PSUM drain is paired into one [128,1024] op on
2-bank PSUM tiles ([128,2,512]) to amortize the fixed cost. Elementwise
work is spread over DVE/ACT/GPSIMD (~14 [128,1024]-ops each per sample):
ACT does squares+exps+half the norms, DVE does reduces+q-drains+o-muls,
GPSIMD does k/v-drains + the other half. Softmax normalization happens in
the o-drain (psum * 1/denom -> fp8), keeping the proj epilogue a single
scalar_tensor_tensor (+pb, +x residual) alternated DVE/GPSIMD to halve
the end-of-kernel tail.

Startup: four fp32 warmup matmuls (gsum x the just-landed x chunks) keep
the PE continuously busy from the first DMA until real work, so the HAM
clock gate reaches K=8/8 (2.4 GHz) before the fp8 stream starts instead
of ~10us into it.

Layouts on chip (per sample):
  hn/q/k/o: channel-major [128, 4, 1024]  (partition = channel % 128)
  v: token-major [128, 8, 512]            (computed transposed by swapping
                                           matmul operands; avoids on-chip
                                           transposes entirely)
  pT = exp(scores^T): [128(token m), 8, 1024(token n)] fp8.
"""

import math
import numpy as np

import concourse.bass as bass
import concourse.bacc as bacc
import concourse.tile as tile
from concourse import bass_isa, mybir
from concourse.bass_utils import run_bass_kernel_spmd

F32 = mybir.dt.float32
FP8 = mybir.dt.float8e4
AF = mybir.ActivationFunctionType
OP = mybir.AluOpType
AX = mybir.AxisListType
DR = mybir.MatmulPerfMode.DoubleRow

B = 16
C = 512
HW = 1024
NCORES = 8
SPC = B // NCORES          # samples per core
KO = C // 128              # channel chunks of 128
KP = KO // 2               # channel pair-chunks (256-deep DoubleRow)
MI = HW // 128             # token chunks of 128
MP = MI // 2               # token pair-chunks
NH = HW // 512             # 512-wide column halves
GSIZE = (C // 32) * HW     # elements per group (16 ch * 1024)
EPS = 1e-5
SM_SCALE = 1.0 / math.sqrt(C)


def build() -> bass.Bass:
    nc = bacc.Bacc()

    x_h = nc.declare_dram_parameter("x", [SPC, C, HW], F32, isOutput=False)
    wq_h = nc.declare_dram_parameter("wq", [C, C], FP8, isOutput=False)
    wk_h = nc.declare_dram_parameter("wk", [C, C], FP8, isOutput=False)
    wv_h = nc.declare_dram_parameter("wv", [C, C], FP8, isOutput=False)
    wp_h = nc.declare_dram_parameter("wp", [C, C], FP8, isOutput=False)
    bq_h = nc.declare_dram_parameter("bq", [C], F32, isOutput=False)
    pb_h = nc.declare_dram_parameter("pb", [C], F32, isOutput=False)
    gam_h = nc.declare_dram_parameter("gam", [C], F32, isOutput=False)
    bet_h = nc.declare_dram_parameter("bet", [C], F32, isOutput=False)
    gs_h = nc.declare_dram_parameter("gsum", [128, 128], F32, isOutput=False)
    on_h = nc.declare_dram_parameter("ones8", [128, 2, 128], FP8, isOutput=False)
    y_h = nc.declare_dram_parameter("y", [SPC, C, HW], F32, isOutput=True)

    with tile.TileContext(nc) as tc:
        with (
            tc.tile_pool(name="const", bufs=1) as const,
            tc.tile_pool(name="xp", bufs=2) as xp,
            tc.tile_pool(name="work", bufs=2) as work,
            tc.tile_pool(name="small", bufs=2) as small,
            tc.tile_pool(name="yp", bufs=3) as yp,
            # 3x 2-bank pair tiles + 2x 1-bank tiles = 8 PSUM banks
            tc.tile_pool(name="psA", bufs=3, space="PSUM") as psA,
            tc.tile_pool(name="psv", bufs=2, space="PSUM") as psv,
        ):
            # x chunk tiles for both samples; sample 0's chunks are DMA'd
            # before everything else so its stats can start immediately
            x_sbs = [[xp.tile([128, HW], F32, tag=f"x{ko}", name=f"x_sb_{s}_{ko}")
                      for ko in range(KO)] for s in range(SPC)]
            for ko in range(KO):
                nc.sync.dma_start(out=x_sbs[0][ko],
                                  in_=x_h[0][ko * 128:(ko + 1) * 128, :])

            # small constants next: the GroupNorm chain needs them long
            # before the big weight tiles are touched
            gs_sb = const.tile([128, 128], F32, tag="gs")
            nc.sync.dma_start(out=gs_sb, in_=gs_h[:])
            bq_sb = const.tile([128, KO], F32, tag="bq")
            nc.sync.dma_start(out=bq_sb, in_=bq_h[:].rearrange("(mo p) -> p mo", p=128))
            pb_sb = const.tile([128, KO], F32, tag="pb")
            nc.sync.dma_start(out=pb_sb, in_=pb_h[:].rearrange("(mo p) -> p mo", p=128))
            gam_sb = const.tile([128, KO], F32, tag="gam")
            nc.sync.dma_start(out=gam_sb, in_=gam_h[:].rearrange("(ko p) -> p ko", p=128))
            bet_sb = const.tile([128, KO], F32, tag="bet")
            nc.sync.dma_start(out=bet_sb, in_=bet_h[:].rearrange("(ko p) -> p ko", p=128))
            ones8_sb = const.tile([128, 2, 128], FP8, tag="ones8")
            nc.sync.dma_start(out=ones8_sb, in_=on_h[:])
            eps_sb = const.tile([128, 1], F32, tag="eps")
            nc.vector.memset(eps_sb, EPS)
            zero_sb = const.tile([128, 1], F32, tag="zero")
            nc.vector.memset(zero_sb, 0.0)
            junk_sb = const.tile([128, HW], F32, tag="junk")

            # HAM warmup: strict-fp32 matmuls (4 PE passes each, ~1.7us) on
            # each x chunk as it lands keep the PE continuously busy through
            # the DMA+stats window, so the clock gate is at K=8/8 before the
            # fp8 stream begins. Results are never read.
            warmjunk_sb = const.tile([128, KO], F32, tag="warmjunk")
            for ko in range(KO):
                warm_ps = psv.tile([128, 512], F32, tag="pv", name=f"warm_{ko}")
                nc.tensor.matmul(warm_ps, lhsT=gs_sb, rhs=x_sbs[0][ko][:, 0:512],
                                 start=True, stop=True)
                # satisfy the BIR verifier (PSUM must have a reader; GPSIMD
                # cannot touch PSUM, so this rides DVE)
                nc.vector.tensor_copy(out=warmjunk_sb[:, ko:ko + 1],
                                      in_=warm_ps[:, 0:1])

            wq_sb = const.tile([128, KO, C], FP8, tag="wq")
            nc.sync.dma_start(out=wq_sb, in_=wq_h[:].rearrange("(ki p) n -> p ki n", p=128))
            wk_sb = const.tile([128, KO, C], FP8, tag="wk")
            nc.sync.dma_start(out=wk_sb, in_=wk_h[:].rearrange("(ki p) n -> p ki n", p=128))
            wv_sb = const.tile([128, KO, C], FP8, tag="wv")
            nc.sync.dma_start(out=wv_sb, in_=wv_h[:].rearrange("(ki p) n -> p ki n", p=128))
            wp_sb = const.tile([128, KO, C], FP8, tag="wp")
            nc.sync.dma_start(out=wp_sb, in_=wp_h[:].rearrange("(ki p) n -> p ki n", p=128))
            # prefetch sample 1
            for ko in range(KO):
                nc.sync.dma_start(out=x_sbs[1][ko],
                                  in_=x_h[1][ko * 128:(ko + 1) * 128, :])

            def emit_gn_stats(s):
                """Per-channel scale/offset for GroupNorm of sample s."""
                x_sb = x_sbs[s]
                st_sb = small.tile([128, KO, 2], F32, tag="st", name=f"st_{s}")
                for ko in range(KO):
                    nc.vector.reduce_sum(out=st_sb[:, ko, 0:1], in_=x_sb[ko], axis=AX.X)
                    # squares land in a scratch tile; only the accumulated
                    # sum-of-squares is kept
                    nc.scalar.activation(
                        out=junk_sb, in_=x_sb[ko],
                        func=AF.Square, bias=zero_sb,
                        accum_out=st_sb[:, ko, 1:2],
                    )
                gps = psv.tile([128, KO, 2], F32, tag="pv", name=f"gps_{s}")
                for ko in range(KO):
                    nc.tensor.matmul(gps[:, ko, :], lhsT=gs_sb, rhs=st_sb[:, ko, :],
                                     start=True, stop=True)
                # gsum is pre-scaled by 1/GSIZE on the host, so gps holds
                # [mean, E[x^2]] directly.
                mean_sb = small.tile([128, KO], F32, tag="mean", name=f"mean_{s}")
                nc.vector.tensor_copy(out=mean_sb, in_=gps[:, :, 0])
                msq_sb = small.tile([128, KO], F32, tag="msq", name=f"msq_{s}")
                nc.vector.tensor_mul(msq_sb, mean_sb, mean_sb)
                var_sb = small.tile([128, KO], F32, tag="var", name=f"var_{s}")
                nc.vector.tensor_sub(var_sb, gps[:, :, 1], msq_sb)
                # rstd = exp(-0.5*ln(var+eps)); same ACT table set as
                # Square/Identity/Exp so no table swap anywhere
                lnv_sb = small.tile([128, KO], F32, tag="lnv", name=f"lnv_{s}")
                nc.scalar.activation(out=lnv_sb, in_=var_sb, func=AF.Ln, bias=eps_sb)
                rstd_sb = small.tile([128, KO], F32, tag="rstd", name=f"rstd_{s}")
                nc.scalar.activation(out=rstd_sb, in_=lnv_sb, func=AF.Exp,
                                     bias=zero_sb, scale=-0.5)
                scl_sb = small.tile([128, KO], F32, tag="scl", name=f"scl_{s}")
                nc.gpsimd.tensor_mul(scl_sb, rstd_sb, gam_sb)
                off_sb = small.tile([128, KO], F32, tag="off", name=f"off_{s}")
                nc.gpsimd.tensor_mul(off_sb, mean_sb, scl_sb)
                nc.gpsimd.tensor_sub(off_sb, bet_sb, off_sb)
                return scl_sb, off_sb

            def emit_gn_norm(s, scl_sb, off_sb, engines):
                """hn = x*scl + off, chunk ko on engines[ko]."""
                hn_sb = work.tile([128, KO, HW], FP8, tag="hn", name=f"hn_{s}")
                for ko in range(KO):
                    eng = engines[ko]
                    if eng == "act":
                        nc.scalar.activation(
                            out=hn_sb[:, ko, :], in_=x_sbs[s][ko],
                            func=AF.Identity, bias=off_sb[:, ko:ko + 1],
                            scale=scl_sb[:, ko:ko + 1])
                    else:
                        e = nc.vector if eng == "dve" else nc.gpsimd
                        e.tensor_scalar(
                            out=hn_sb[:, ko, :], in0=x_sbs[s][ko],
                            scalar1=scl_sb[:, ko:ko + 1], scalar2=off_sb[:, ko:ko + 1],
                            op0=OP.mult, op1=OP.add,
                        )
                return hn_sb

            def emit_qkv(s, hn_sb):
                q_sb = work.tile([128, KO, HW], FP8, tag="q", name=f"q_{s}")
                k_sb = work.tile([128, KO, HW], FP8, tag="k", name=f"k_{s}")
                v_sb = work.tile([128, MI, C], FP8, tag="v", name=f"v_{s}")
                for mo in range(KO):
                    msl = slice(mo * 128, (mo + 1) * 128)
                    pq = psA.tile([128, NH, 512], F32, tag="pmm", name="pq")
                    for j in range(KP):
                        for nh in range(NH):
                            nc.tensor.matmul(
                                pq[:, nh, :], lhsT=wq_sb[:, 2 * j:2 * j + 2, msl],
                                rhs=hn_sb[:, 2 * j:2 * j + 2, nh * 512:(nh + 1) * 512],
                                start=(j == 0), stop=(j == KP - 1), perf_mode=DR)
                    nc.vector.tensor_scalar_add(out=q_sb[:, mo, :], in0=pq,
                                                scalar1=bq_sb[:, mo:mo + 1])
                    pk = psA.tile([128, NH, 512], F32, tag="pmm", name="pk")
                    for j in range(KP):
                        for nh in range(NH):
                            nc.tensor.matmul(
                                pk[:, nh, :], lhsT=wk_sb[:, 2 * j:2 * j + 2, msl],
                                rhs=hn_sb[:, 2 * j:2 * j + 2, nh * 512:(nh + 1) * 512],
                                start=(j == 0), stop=(j == KP - 1), perf_mode=DR)
                    nc.scalar.copy(out=k_sb[:, mo, :], in_=pk)
                # v: two token-chunks share one 2-bank psum tile so the
                # drain is a single [128,1024] op
                for u in range(MP):
                    pvv = psA.tile([128, 2, 512], F32, tag="pmm", name="pvv")
                    for t in range(2):
                        mi = 2 * u + t
                        for j in range(KP):
                            nc.tensor.matmul(
                                pvv[:, t, :],
                                lhsT=hn_sb[:, 2 * j:2 * j + 2, mi * 128:(mi + 1) * 128],
                                rhs=wv_sb[:, 2 * j:2 * j + 2, :],
                                start=(j == 0), stop=(j == KP - 1), perf_mode=DR)
                    if u % 2 == 0:
                        nc.vector.tensor_copy(out=v_sb[:, 2 * u:2 * u + 2, :], in_=pvv)
                    else:
                        nc.scalar.copy(out=v_sb[:, 2 * u:2 * u + 2, :], in_=pvv)
                return q_sb, k_sb, v_sb

            def emit_attention(s, q_sb, k_sb, v_sb):
                pT_sb = work.tile([128, MI, HW], FP8, tag="pT", name=f"pT_{s}")
                rbc_sb = small.tile([128, HW], F32, tag="rbc", name=f"rbc_{s}")
                for mi in range(MI):
                    sps = psA.tile([128, NH, 512], F32, tag="pmm", name="sps")
                    for j in range(KP):
                        for nh in range(NH):
                            nc.tensor.matmul(
                                sps[:, nh, :],
                                lhsT=k_sb[:, 2 * j:2 * j + 2, mi * 128:(mi + 1) * 128],
                                rhs=q_sb[:, 2 * j:2 * j + 2, nh * 512:(nh + 1) * 512],
                                start=(j == 0), stop=(j == KP - 1), perf_mode=DR)
                    nc.scalar.activation(out=pT_sb[:, mi, :], in_=sps,
                                         func=AF.Exp, bias=zero_sb, scale=SM_SCALE)
                # softmax denominators for both halves: fp8 all-ones
                # DoubleRow matmuls reduce pT across tokens and broadcast to
                # all 128 PSUM partitions; reciprocals hide under the first
                # attn@V group's PE time.
                dps = [psv.tile([128, 512], F32, tag="pv", name=f"dps{nh}_{s}")
                       for nh in range(NH)]
                for nh in range(NH):
                    for u in range(MP):
                        nc.tensor.matmul(
                            dps[nh], lhsT=ones8_sb,
                            rhs=pT_sb[:, 2 * u:2 * u + 2, nh * 512:(nh + 1) * 512],
                            start=(u == 0), stop=(u == MP - 1), perf_mode=DR)
                o_sb = work.tile([128, KO, HW], FP8, tag="o", name=f"o_{s}")
                for co in range(KO):
                    ops = psA.tile([128, NH, 512], F32, tag="pmm", name="ops")
                    for u in range(MP):
                        for nh in range(NH):
                            nc.tensor.matmul(
                                ops[:, nh, :],
                                lhsT=v_sb[:, 2 * u:2 * u + 2, co * 128:(co + 1) * 128],
                                rhs=pT_sb[:, 2 * u:2 * u + 2, nh * 512:(nh + 1) * 512],
                                start=(u == 0), stop=(u == MP - 1), perf_mode=DR)
                    if co == 0:
                        for nh in range(NH):
                            nc.vector.reciprocal_approx_fast(
                                out=rbc_sb[:, nh * 512:(nh + 1) * 512], in_=dps[nh])
                    # normalization happens in the drain: o = psum * (1/denom)
                    nc.vector.tensor_mul(o_sb[:, co, :], ops, rbc_sb)
                return o_sb

            def emit_proj(s, o_sb):
                for co in range(KO):
                    pp = psA.tile([128, NH, 512], F32, tag="pmm", name="pp")
                    for j in range(KP):
                        for nh in range(NH):
                            nc.tensor.matmul(
                                pp[:, nh, :],
                                lhsT=wp_sb[:, 2 * j:2 * j + 2, co * 128:(co + 1) * 128],
                                rhs=o_sb[:, 2 * j:2 * j + 2, nh * 512:(nh + 1) * 512],
                                start=(j == 0), stop=(j == KP - 1), perf_mode=DR)
                    y_sb = yp.tile([128, HW], F32, tag="y", name="y_sb")
                    if co % 2 == 0:
                        # even chunks: one fused DVE op (psum + pb) + x
                        nc.vector.scalar_tensor_tensor(
                            out=y_sb, in0=pp, scalar=pb_sb[:, co:co + 1],
                            in1=x_sbs[s][co], op0=OP.add, op1=OP.add)
                    else:
                        # odd chunks: ACT drains psum (+pb), GPSIMD adds the
                        # residual (SBUF-only) -- pipelines the epilogue tail
                        # across three engines
                        t_sb = yp.tile([128, HW], F32, tag="t", name="t_sb")
                        nc.scalar.activation(out=t_sb, in_=pp, func=AF.Identity,
                                             bias=pb_sb[:, co:co + 1])
                        nc.gpsimd.tensor_add(y_sb, t_sb, x_sbs[s][co])
                    nc.sync.dma_start(out=y_h[s][co * 128:(co + 1) * 128, :], in_=y_sb)

            # software-pipelined schedule over the two samples
            scl0, off0 = emit_gn_stats(0)
            hn0 = emit_gn_norm(0, scl0, off0, ["dve", "act", "gpsimd", "dve"])
            qkv0 = emit_qkv(0, hn0)
            # sample 1 stats ride the idle DVE/ACT time under sample 0's
            # attention; its normalize runs during sample 0's o/proj matmuls
            scl1, off1 = emit_gn_stats(1)
            o0 = emit_attention(0, *qkv0)
            hn1 = emit_gn_norm(1, scl1, off1, ["gpsimd", "gpsimd", "gpsimd", "gpsimd"])
            emit_proj(0, o0)
            qkv1 = emit_qkv(1, hn1)
            o1 = emit_attention(1, *qkv1)
            emit_proj(1, o1)

    nc.compile()
    return nc


_NC_CACHE: dict = {}


def _get_nc() -> bass.Bass:
    if "fp8" not in _NC_CACHE:
        _NC_CACHE["fp8"] = build()
    return _NC_CACHE["fp8"]


def make_in_maps(x, gamma, beta, qkv_w, qkv_b, proj_w, proj_b):
    import ml_dtypes
    f32 = np.float32
    fp8 = np.dtype(ml_dtypes.float8_e4m3)
    x = np.ascontiguousarray(np.asarray(x, dtype=f32)).reshape(B, C, HW)
    qkv_w = np.asarray(qkv_w, dtype=f32)
    qkv_b = np.asarray(qkv_b, dtype=f32)
    proj_w = np.asarray(proj_w, dtype=f32)
    proj_b = np.asarray(proj_b, dtype=f32)
    shared = {
        "wq": np.ascontiguousarray(qkv_w[0:C].T).astype(fp8),
        "wk": np.ascontiguousarray(qkv_w[C:2 * C].T).astype(fp8),
        "wv": np.ascontiguousarray(qkv_w[2 * C:3 * C].T).astype(fp8),
        "wp": np.ascontiguousarray(proj_w.T).astype(fp8),
        "bq": np.ascontiguousarray(qkv_b[0:C]),
        "pb": (proj_w.astype(np.float64) @ qkv_b[2 * C:3 * C].astype(np.float64)
               + proj_b.astype(np.float64)).astype(f32),
        "gam": np.ascontiguousarray(np.asarray(gamma, dtype=f32)),
        "bet": np.ascontiguousarray(np.asarray(beta, dtype=f32)),
        "gsum": np.kron(np.eye(8, dtype=f32), np.ones((16, 16), dtype=f32)) * f32(1.0 / GSIZE),
        "ones8": np.ones((128, 2, 128), dtype=fp8),
    }
    return [dict(shared, x=np.ascontiguousarray(x[i * SPC:(i + 1) * SPC]))
            for i in range(NCORES)]


def run(x, gamma, beta, qkv_w, qkv_b, proj_w, proj_b, trace=False, dtype_mode="fp8"):
    in_maps = make_in_maps(x, gamma, beta, qkv_w, qkv_b, proj_w, proj_b)
    nc = _get_nc()
    res = run_bass_kernel_spmd(nc, in_maps, list(range(NCORES)), trace=trace)
    y = np.concatenate([res.results[i]["y"] for i in range(NCORES)], axis=0)
    return y.reshape(B, C, 32, 32).astype(np.float32), res


def kernel(**inputs) -> np.ndarray:
    y, _ = run(**inputs)
    return y
